# revision 1
# baseline (speedup 1.0000x reference)
"""AudioSNN Trainium2 kernel.

Two-layer leaky-integrate-and-fire SNN (snntorch Leaky, reset-by-subtract),
T=500 recurrent steps over batch 4096, data-parallel over 8 NeuronCores
(512 batch elements per core).

Math (per step t, reference):
    cur1 = x_t @ W1.T + b1
    m1   = beta*m1 + cur1 - H(m1_prev - 1)        # H(m1_prev-1) == spk1[t-1]
    spk1 = H(m1 - 1)
    cur2 = spk1 @ W2.T + b2
    m2   = beta*m2 + cur2 - spk2[t-1]
    spk2 = H(m2 - 1)    -> output [T, B, 5]

Device formulation (per core, full 512-batch tiles, all fp32):
  L1 state z1 = m1 - 1, spikes in sign form sgn = sign(z1) (spk = (sgn+1)/2):
    psum1 = W1aug^T.T @ x_aug     (bias b1+beta-1.5 via ones-row of x_aug)
    psum1 += (-0.5*I128) @ sgn[t-1]   (= -spk1[t-1] + const folded into bias)
    z1[t] = beta*z1[t-1] + psum1  (one DVE scalar_tensor_tensor)
    sgn[t] = Sign(z1[t])          (ACT engine)
  L2 state z2 = m2 - 1 - p, p = C2/(1-beta), C2 = 0.5*sum(W2,h) + b2 + beta-1:
    psum2 = (0.5*W2^T).T @ sgn[t] -> [5, 512]
    psum2 += (-I5) @ spk2[t-1]
    z2[t] = beta*z2[t-1] + psum2  (DVE)
    spk2[t] = (z2[t] > -p)        (DVE tensor_single_scalar, per-partition -p)
"""

import os
import sys

sys.path.insert(0, "/opt/trn_rl_repo")

from contextlib import ExitStack

import numpy as np

from concourse import bacc, mybir, tile
from concourse.bass_utils import run_bass_kernel_spmd

BETA = 0.9
T, F, H, O = 500, 40, 128, 5
NCORES = 8
BC = 512  # batch per core
CH = 20  # time steps per DMA chunk (must divide T)
F32 = mybir.dt.float32

MULT = mybir.AluOpType.mult
ADD = mybir.AluOpType.add
IS_GT = mybir.AluOpType.is_gt


def build(nc, n_steps=T, ch=CH, cut_pe_edges=False, variant="full"):
    """Emit the per-core program. x_aug layout: [n_chunks, (F+1)*ch*BC]."""
    n_chunks = n_steps // ch

    x_d = nc.dram_tensor(
        "x_aug", [n_chunks, (F + 1) * ch * BC], F32, kind="ExternalInput"
    ).ap()
    w1_d = nc.dram_tensor("w1aug", [F + 1, H], F32, kind="ExternalInput").ap()
    nhi_d = nc.dram_tensor("neg_half_i", [H, H], F32, kind="ExternalInput").ap()
    ni5_d = nc.dram_tensor("neg_i5", [O, O], F32, kind="ExternalInput").ap()
    w2h_d = nc.dram_tensor("w2half", [H, O], F32, kind="ExternalInput").ap()
    npp_d = nc.dram_tensor("negp", [O, 1], F32, kind="ExternalInput").ap()
    z2i_d = nc.dram_tensor("z2init", [O, BC], F32, kind="ExternalInput").ap()
    out_d = nc.dram_tensor("out", [O, n_steps * BC], F32, kind="ExternalOutput").ap()

    with tile.TileContext(nc) as tc, ExitStack() as ctx:
        const = ctx.enter_context(tc.tile_pool(name="const", bufs=1))
        state = ctx.enter_context(tc.tile_pool(name="state", bufs=1))
        xin = ctx.enter_context(tc.tile_pool(name="xin", bufs=2))
        outp = ctx.enter_context(tc.tile_pool(name="outp", bufs=2))
        ps1 = ctx.enter_context(tc.tile_pool(name="ps1", bufs=4, space="PSUM"))
        ps2 = ctx.enter_context(tc.tile_pool(name="ps2", bufs=4, space="PSUM"))

        w1_s = const.tile([F + 1, H], F32, tag="w1")
        nhi_s = const.tile([H, H], F32, tag="nhi")
        ni5_s = const.tile([O, O], F32, tag="ni5")
        w2h_s = const.tile([H, O], F32, tag="w2h")
        npp_s = const.tile([O, 1], F32, tag="npp")
        for s, d in [
            (w1_s, w1_d),
            (nhi_s, nhi_d),
            (ni5_s, ni5_d),
            (w2h_s, w2h_d),
            (npp_s, npp_d),
        ]:
            nc.sync.dma_start(out=s[:], in_=d[:])

        # Recurrent state, ping-pong buffered (index = t % 2).
        z1 = [state.tile([H, BC], F32, tag=f"z1_{pp}", name=f"z1_{pp}") for pp in range(2)]
        sg = [state.tile([H, BC], F32, tag=f"sg_{pp}", name=f"sg_{pp}") for pp in range(2)]
        z2 = [state.tile([O, BC], F32, tag=f"z2_{pp}", name=f"z2_{pp}") for pp in range(2)]
        spk0 = state.tile([O, BC], F32, tag="spk0")

        nc.vector.memset(z1[1][:], -1.0)  # m1(0)=0 -> z1=-1
        nc.vector.memset(sg[1][:], -1.0)  # sign(-1)
        nc.sync.dma_start(out=z2[1][:], in_=z2i_d[:])
        nc.vector.memset(spk0[:], 0.0)

        xt = None
        ot = None
        spk_prev = spk0[:]
        for t in range(n_steps):
            chk, st = divmod(t, ch)
            if st == 0:
                xt = xin.tile([F + 1, ch * BC], F32, tag="xt")
                nc.sync.dma_start(out=xt[:], in_=x_d[chk : chk + 1, :])
                ot = outp.tile([O, ch * BC], F32, tag="ot")
            cur, prv = t % 2, 1 - (t % 2)

            # ---- layer 1 ----
            p1 = ps1.tile([H, BC], F32, tag="p1")
            xs = xt[:, st * BC : (st + 1) * BC]
            nc.tensor.matmul(p1[:], w1_s[:], xs, start=True, stop=False)
            nc.tensor.matmul(
                p1[:], nhi_s[:], sg[1][:] if cut_pe_edges else sg[prv][:],
                start=False, stop=True,
            )
            if variant == "copystt":
                nc.vector.tensor_copy(z1[cur][:], p1[:])
            else:
                nc.vector.scalar_tensor_tensor(
                    z1[cur][:], z1[prv][:], BETA, p1[:], MULT, ADD
                )
            nc.scalar.sign(sg[cur][:], z1[cur][:])

            # ---- layer 2 ([5, 512]) ----
            if variant == "nol2":
                o_slice = ot[:, st * BC : (st + 1) * BC]
                nc.vector.tensor_single_scalar(o_slice, z1[cur][:O, :], npp_s[:], IS_GT)
                if st == ch - 1:
                    nc.sync.dma_start(
                        out=out_d[:, chk * ch * BC : (chk + 1) * ch * BC], in_=ot[:]
                    )
                continue
            p2 = ps2.tile([O, BC], F32, tag="p2")
            nc.tensor.matmul(
                p2[:], w2h_s[:], sg[1][:] if cut_pe_edges else sg[cur][:],
                start=True, stop=False,
            )
            nc.tensor.matmul(
                p2[:], ni5_s[:], spk0[:] if cut_pe_edges else spk_prev,
                start=False, stop=True,
            )
            if variant == "copystt":
                nc.vector.tensor_copy(z2[cur][:], p2[:])
            else:
                nc.vector.scalar_tensor_tensor(
                    z2[cur][:], z2[prv][:], BETA, p2[:], MULT, ADD
                )
            o_slice = ot[:, st * BC : (st + 1) * BC]
            nc.vector.tensor_single_scalar(o_slice, z2[cur][:], npp_s[:], IS_GT)
            spk_prev = o_slice

            if st == ch - 1:
                nc.sync.dma_start(
                    out=out_d[:, chk * ch * BC : (chk + 1) * ch * BC], in_=ot[:]
                )


def host_inputs(x, W1, b1, W2, b2, n_steps=T, ch=CH):
    """Shard + precompute all per-core device input arrays."""
    n_chunks = n_steps // ch
    x = np.asarray(x, np.float32)[:, :n_steps, :]
    W1 = np.asarray(W1, np.float32)
    b1 = np.asarray(b1, np.float32)
    W2 = np.asarray(W2, np.float32)
    b2 = np.asarray(b2, np.float32)

    # x: [B, T', F] -> per core [T', F, 512] -> augment ones -> chunked
    xs = x.reshape(NCORES, BC, n_steps, F).transpose(0, 2, 3, 1)  # [8,T',40,512]
    aug = np.empty((NCORES, n_steps, F + 1, BC), np.float32)
    aug[:, :, :F, :] = xs
    aug[:, :, F, :] = 1.0
    # [8, T', 41, 512] -> [8, n_chunks, 41, ch, 512] (chunk-major, partition dim 41)
    aug = aug.reshape(NCORES, n_chunks, ch, F + 1, BC).transpose(0, 1, 3, 2, 4)
    aug = np.ascontiguousarray(aug).reshape(NCORES, n_chunks, (F + 1) * ch * BC)

    w1aug = np.concatenate([W1.T, (b1 + BETA - 1.5)[None, :]], axis=0)  # [41,128]

    neg_half_i = (-0.5 * np.eye(H)).astype(np.float32)
    neg_i5 = (-np.eye(O)).astype(np.float32)
    w2half = (0.5 * W2.T).astype(np.float32)  # [128, 5]

    s2 = 0.5 * W2.sum(axis=1)  # [5]
    C2 = s2 + b2 + BETA - 1.0
    p = (C2 / (1.0 - BETA)).astype(np.float32)
    negp = (-p)[:, None].astype(np.float32)  # [5, 1]
    z2init = np.tile((-1.0 - p)[:, None], (1, BC)).astype(np.float32)  # [5, 512]

    shared = {
        "w1aug": np.ascontiguousarray(w1aug),
        "neg_half_i": neg_half_i,
        "neg_i5": neg_i5,
        "w2half": w2half,
        "negp": negp,
        "z2init": z2init,
    }
    return [{"x_aug": aug[c], **shared} for c in range(NCORES)]


def assemble(results, n_steps=T):
    """[O, T'*BC] f32 per core -> [T', B, O] float32."""
    outs = []
    for r in results:
        a = np.asarray(r["out"]).reshape(O, n_steps, BC).astype(np.float32)
        outs.append(a.transpose(1, 2, 0))  # [T', 512, 5]
    return np.concatenate(outs, axis=1)


LAST_RESULT = None  # BassKernelResults of the most recent run (for profiling)


def kernel(x, W1, b1, W2, b2):
    global LAST_RESULT
    in_maps = host_inputs(x, W1, b1, W2, b2)
    nc = bacc.Bacc("TRN2", target_bir_lowering=False, debug=False)
    build(nc)
    nc.compile()
    LAST_RESULT = run_bass_kernel_spmd(nc, in_maps, list(range(NCORES)))
    return assemble(LAST_RESULT.results)



# revision 4
# speedup vs baseline: 2.1297x; 2.1297x over previous
"""AudioSNN Trainium2 kernel (v2: fp16-split matmuls).

Two-layer leaky-integrate-and-fire SNN (snntorch Leaky, reset-by-subtract),
T=500 recurrent steps over batch 4096, data-parallel over 8 NeuronCores
(512 batch elements per core).

Math (per step t, reference):
    cur1 = x_t @ W1.T + b1
    m1   = beta*m1 + cur1 - spk1[t-1]
    spk1 = H(m1 - 1)
    cur2 = spk1 @ W2.T + b2
    m2   = beta*m2 + cur2 - spk2[t-1]
    spk2 = H(m2 - 1)    -> output [T, B, 5]

Device formulation (per core, 512-batch tiles, fp16 matmuls / f32 state):
  All matmul operands are fp16 with hi/lo splitting so products stay
  f32-exact (~1e-7): x = xhi + xlo, W1 = W1hi + W1lo, dropping the
  xlo*W1lo term. Spikes are carried in sign form sgn = sign(z) in {-1,+1}
  (exact in fp16), converting reset-subtract to -0.5*sgn + const.

  L1 state z1 = m1 - 1:
    p1 = W1hi^T@xhi + biashi  (mm a, K=41: ones-row pairs with bias)
       + W1lo^T@xhi + W1hi^T@xlo + biaslo  (mm c, K=81)
       + (-0.5 I128)@sg1[t-1]  (mm b)
    z1[t] = beta*z1[t-1] + p1          (DVE scalar_tensor_tensor)
    sg1[t] = Sign(z1[t])  fp16         (ACT)
  L2 state y2 = m2 - 1 - rho, rho = c2/(1-beta),
  c2 = 0.5*sum(W2,h) + b2 + beta - 1.5:
    p2 = (0.5 W2hi)^T@sg1[t] + (0.5 W2lo)^T@sg1[t] + (-0.5 I5)@sg2[t-1]
    y2[t] = beta*y2[t-1] + p2          (DVE)
    sg2[t] = Sign(y2[t] + rho)  fp16   (ACT, per-partition bias)
  Output is sg2 (fp16 sign form); host maps (sg>0) -> 1.0.
"""

import os
import sys

sys.path.insert(0, "/opt/trn_rl_repo")

from contextlib import ExitStack

import numpy as np

from concourse import bacc, mybir, tile
from concourse.bass_utils import run_bass_kernel_spmd

BETA = 0.9
T, F, H, O = 500, 40, 128, 5
NCORES = 8
BC = 512  # batch per core
CH = 20  # time steps per DMA chunk (must divide T)
KA = F + 1  # 41: [xhi; ones]
KC = 2 * F + 1  # 81: [xhi; ones; xlo]
F32 = mybir.dt.float32
F16 = mybir.dt.float16

MULT = mybir.AluOpType.mult
ADD = mybir.AluOpType.add


def build(nc, n_steps=T, ch=CH):
    """Emit the per-core program. x layout: [n_chunks, KC*ch*BC] fp16."""
    n_chunks = n_steps // ch

    x_d = nc.dram_tensor(
        "x_aug", [n_chunks, KC * ch * BC], F16, kind="ExternalInput"
    ).ap()
    w1a_d = nc.dram_tensor("w1a", [KA, H], F16, kind="ExternalInput").ap()
    w1c_d = nc.dram_tensor("w1c", [KC, H], F16, kind="ExternalInput").ap()
    nhi_d = nc.dram_tensor("neg_half_i", [H, H], F16, kind="ExternalInput").ap()
    ni5_d = nc.dram_tensor("neg_i5", [O, O], F16, kind="ExternalInput").ap()
    w2hi_d = nc.dram_tensor("w2hi", [H, O], F16, kind="ExternalInput").ap()
    w2lo_d = nc.dram_tensor("w2lo", [H, O], F16, kind="ExternalInput").ap()
    rho_d = nc.dram_tensor("rho", [O, 1], F32, kind="ExternalInput").ap()
    y2i_d = nc.dram_tensor("y2init", [O, BC], F32, kind="ExternalInput").ap()
    out_d = nc.dram_tensor("out", [O, n_steps * BC], F16, kind="ExternalOutput").ap()

    with tile.TileContext(nc) as tc, ExitStack() as ctx:
        const = ctx.enter_context(tc.tile_pool(name="const", bufs=1))
        state = ctx.enter_context(tc.tile_pool(name="state", bufs=1))
        xin = ctx.enter_context(tc.tile_pool(name="xin", bufs=2))
        outp = ctx.enter_context(tc.tile_pool(name="outp", bufs=2))
        ps1 = ctx.enter_context(tc.tile_pool(name="ps1", bufs=3, space="PSUM"))
        ps2 = ctx.enter_context(tc.tile_pool(name="ps2", bufs=3, space="PSUM"))

        w1a_s = const.tile([KA, H], F16, tag="w1a")
        w1c_s = const.tile([KC, H], F16, tag="w1c")
        nhi_s = const.tile([H, H], F16, tag="nhi")
        ni5_s = const.tile([O, O], F16, tag="ni5")
        w2hi_s = const.tile([H, O], F16, tag="w2hi")
        w2lo_s = const.tile([H, O], F16, tag="w2lo")
        rho_s = const.tile([O, 1], F32, tag="rho")
        for s, d in [
            (w1a_s, w1a_d),
            (w1c_s, w1c_d),
            (nhi_s, nhi_d),
            (ni5_s, ni5_d),
            (w2hi_s, w2hi_d),
            (w2lo_s, w2lo_d),
            (rho_s, rho_d),
        ]:
            nc.sync.dma_start(out=s[:], in_=d[:])

        # Recurrent state, ping-pong buffered (index = t % 2).
        z1 = [state.tile([H, BC], F32, tag=f"z1_{p}", name=f"z1_{p}") for p in range(2)]
        sg = [state.tile([H, BC], F16, tag=f"sg_{p}", name=f"sg_{p}") for p in range(2)]
        y2 = [state.tile([O, BC], F32, tag=f"y2_{p}", name=f"y2_{p}") for p in range(2)]
        sg2init = state.tile([O, BC], F16, tag="sg2init")

        nc.vector.memset(z1[1][:], -1.0)  # m1(0)=0 -> z1=-1
        nc.vector.memset(sg[1][:], -1.0)  # sign(-1)
        nc.sync.dma_start(out=y2[1][:], in_=y2i_d[:])
        nc.vector.memset(sg2init[:], -1.0)

        # x-chunk DMA row split across the 3 DMA-capable queues
        dma_engines = [nc.sync, nc.gpsimd, nc.scalar]
        row_splits = [0, 27, 54, KC]
        rl = ch * BC  # elements per row in a chunk

        xt = None
        ot = None
        sg2_prev = sg2init[:]
        for t in range(n_steps):
            chk, st = divmod(t, ch)
            if st == 0:
                xt = xin.tile([KC, ch * BC], F16, tag="xt")
                for q in range(3):
                    r0, r1 = row_splits[q], row_splits[q + 1]
                    dma_engines[q].dma_start(
                        out=xt[r0:r1, :],
                        in_=x_d[chk : chk + 1, r0 * rl : r1 * rl],
                    )
                ot = outp.tile([O, ch * BC], F16, tag="ot")
            cur, prv = t % 2, 1 - (t % 2)

            # ---- layer 1 ----
            p1 = ps1.tile([H, BC], F32, tag="p1")
            xs_a = xt[:KA, st * BC : (st + 1) * BC]
            xs_c = xt[:KC, st * BC : (st + 1) * BC]
            nc.tensor.matmul(p1[:], w1a_s[:], xs_a, start=True, stop=False)
            nc.tensor.matmul(p1[:], w1c_s[:], xs_c, start=False, stop=False)
            nc.tensor.matmul(p1[:], nhi_s[:], sg[prv][:], start=False, stop=True)
            nc.vector.scalar_tensor_tensor(
                z1[cur][:], z1[prv][:], BETA, p1[:], MULT, ADD
            )
            nc.scalar.sign(sg[cur][:], z1[cur][:])

            # ---- layer 2 ([5, 512]) ----
            p2 = ps2.tile([O, BC], F32, tag="p2")
            nc.tensor.matmul(p2[:], w2hi_s[:], sg[cur][:], start=True, stop=False)
            nc.tensor.matmul(p2[:], w2lo_s[:], sg[cur][:], start=False, stop=False)
            nc.tensor.matmul(p2[:], ni5_s[:], sg2_prev, start=False, stop=True)
            nc.vector.scalar_tensor_tensor(
                y2[cur][:], y2[prv][:], BETA, p2[:], MULT, ADD
            )
            o_slice = ot[:, st * BC : (st + 1) * BC]
            nc.scalar.sign(o_slice, y2[cur][:], bias=rho_s[:])
            sg2_prev = o_slice

            if st == ch - 1:
                nc.sync.dma_start(
                    out=out_d[:, chk * ch * BC : (chk + 1) * ch * BC], in_=ot[:]
                )


def _split16(a):
    hi = a.astype(np.float16)
    lo = (a.astype(np.float32) - hi.astype(np.float32)).astype(np.float16)
    return hi, lo


def host_inputs(x, W1, b1, W2, b2, n_steps=T, ch=CH):
    """Shard + precompute all per-core device input arrays."""
    n_chunks = n_steps // ch
    x = np.asarray(x, np.float32)[:, :n_steps, :]
    W1 = np.asarray(W1, np.float32)
    b1 = np.asarray(b1, np.float32)
    W2 = np.asarray(W2, np.float32)
    b2 = np.asarray(b2, np.float32)

    # x: [B, T', F] -> per core [T', F, 512] hi/lo-split + ones row
    xs = x.reshape(NCORES, BC, n_steps, F).transpose(0, 2, 3, 1)  # [8,T',40,512]
    xhi, xlo = _split16(xs)
    aug = np.empty((NCORES, n_steps, KC, BC), np.float16)
    aug[:, :, :F, :] = xhi
    aug[:, :, F, :] = 1.0
    aug[:, :, F + 1 :, :] = xlo
    # -> [8, n_chunks, 81, ch, 512] (chunk-major, partition dim 81)
    aug = aug.reshape(NCORES, n_chunks, ch, KC, BC).transpose(0, 1, 3, 2, 4)
    aug = np.ascontiguousarray(aug).reshape(NCORES, n_chunks, KC * ch * BC)

    w1hi, w1lo = _split16(W1.T)  # [40, 128]
    bias1 = b1 + BETA - 1.5
    bhi, blo = _split16(bias1)
    w1a = np.concatenate([w1hi, bhi[None, :]], axis=0)  # [41, 128]
    w1c = np.concatenate([w1lo, blo[None, :], w1hi], axis=0)  # [81, 128]

    neg_half_i = (-0.5 * np.eye(H)).astype(np.float16)
    neg_i5 = (-0.5 * np.eye(O)).astype(np.float16)
    w2hi, w2lo = _split16(0.5 * W2.T)  # [128, 5]

    c2 = 0.5 * W2.sum(axis=1) + b2 + BETA - 1.5
    rho = (c2 / (1.0 - BETA)).astype(np.float32)
    y2init = np.tile((-1.0 - rho)[:, None], (1, BC)).astype(np.float32)

    shared = {
        "w1a": np.ascontiguousarray(w1a),
        "w1c": np.ascontiguousarray(w1c),
        "neg_half_i": neg_half_i,
        "neg_i5": neg_i5,
        "w2hi": np.ascontiguousarray(w2hi),
        "w2lo": np.ascontiguousarray(w2lo),
        "rho": rho[:, None].copy(),
        "y2init": y2init,
    }
    return [{"x_aug": aug[c], **shared} for c in range(NCORES)]


def assemble(results, n_steps=T):
    """[O, T'*BC] fp16 sign form per core -> [T', B, O] float32 spikes."""
    outs = []
    for r in results:
        a = np.asarray(r["out"]).reshape(O, n_steps, BC)
        outs.append((a > 0).transpose(1, 2, 0).astype(np.float32))  # [T', 512, 5]
    return np.concatenate(outs, axis=1)


LAST_RESULT = None  # BassKernelResults of the most recent run (for profiling)


def kernel(x, W1, b1, W2, b2):
    global LAST_RESULT
    in_maps = host_inputs(x, W1, b1, W2, b2)
    nc = bacc.Bacc("TRN2", target_bir_lowering=False, debug=False)
    build(nc)
    nc.compile()
    LAST_RESULT = run_bass_kernel_spmd(nc, in_maps, list(range(NCORES)))
    return assemble(LAST_RESULT.results)


# revision 5
# speedup vs baseline: 2.1298x; 1.0001x over previous
"""AudioSNN Trainium2 kernel (v2: fp16-split matmuls).

Two-layer leaky-integrate-and-fire SNN (snntorch Leaky, reset-by-subtract),
T=500 recurrent steps over batch 4096, data-parallel over 8 NeuronCores
(512 batch elements per core).

Math (per step t, reference):
    cur1 = x_t @ W1.T + b1
    m1   = beta*m1 + cur1 - spk1[t-1]
    spk1 = H(m1 - 1)
    cur2 = spk1 @ W2.T + b2
    m2   = beta*m2 + cur2 - spk2[t-1]
    spk2 = H(m2 - 1)    -> output [T, B, 5]

Device formulation (per core, 512-batch tiles, fp16 matmuls / f32 state):
  All matmul operands are fp16 with hi/lo splitting so products stay
  f32-exact (~1e-7): x = xhi + xlo, W1 = W1hi + W1lo, dropping the
  xlo*W1lo term. Spikes are carried in sign form sgn = sign(z) in {-1,+1}
  (exact in fp16), converting reset-subtract to -0.5*sgn + const.

  L1 state z1 = m1 - 1:
    p1 = W1hi^T@xhi + biashi  (mm a, K=41: ones-row pairs with bias)
       + W1lo^T@xhi + W1hi^T@xlo + biaslo  (mm c, K=81)
       + (-0.5 I128)@sg1[t-1]  (mm b)
    z1[t] = beta*z1[t-1] + p1          (DVE scalar_tensor_tensor)
    sg1[t] = Sign(z1[t])  fp16         (ACT)
  L2 state y2 = m2 - 1 - rho, rho = c2/(1-beta),
  c2 = 0.5*sum(W2,h) + b2 + beta - 1.5:
    p2 = (0.5 W2hi)^T@sg1[t] + (0.5 W2lo)^T@sg1[t] + (-0.5 I5)@sg2[t-1]
    y2[t] = beta*y2[t-1] + p2          (DVE)
    sg2[t] = Sign(y2[t] + rho)  fp16   (ACT, per-partition bias)
  Output is sg2 (fp16 sign form); host maps (sg>0) -> 1.0.
"""

import os
import sys

sys.path.insert(0, "/opt/trn_rl_repo")

from contextlib import ExitStack

import numpy as np

from concourse import bacc, mybir, tile
from concourse.bass_utils import run_bass_kernel_spmd

BETA = 0.9
T, F, H, O = 500, 40, 128, 5
NCORES = 8
BC = 512  # batch per core
CH = 20  # time steps per DMA chunk (must divide T)
KA = F + 1  # 41: [xhi; ones]
KC = 2 * F + 1  # 81: [xhi; ones; xlo]
F32 = mybir.dt.float32
F16 = mybir.dt.float16

MULT = mybir.AluOpType.mult
ADD = mybir.AluOpType.add


def build(nc, n_steps=T, ch=CH):
    """Emit the per-core program. x layout: [n_chunks, KC*ch*BC] fp16."""
    n_chunks = n_steps // ch

    x_d = nc.dram_tensor(
        "x_aug", [n_chunks, KC * ch * BC], F16, kind="ExternalInput"
    ).ap()
    w1a_d = nc.dram_tensor("w1a", [KA, H], F16, kind="ExternalInput").ap()
    w1c_d = nc.dram_tensor("w1c", [KC, H], F16, kind="ExternalInput").ap()
    nhi_d = nc.dram_tensor("neg_half_i", [H, H], F16, kind="ExternalInput").ap()
    ni5_d = nc.dram_tensor("neg_i5", [O, O], F16, kind="ExternalInput").ap()
    w2hi_d = nc.dram_tensor("w2hi", [H, O], F16, kind="ExternalInput").ap()
    w2lo_d = nc.dram_tensor("w2lo", [H, O], F16, kind="ExternalInput").ap()
    rho_d = nc.dram_tensor("rho", [O, 1], F32, kind="ExternalInput").ap()
    y2i_d = nc.dram_tensor("y2init", [O, BC], F32, kind="ExternalInput").ap()
    out_d = nc.dram_tensor("out", [O, n_steps * BC], F16, kind="ExternalOutput").ap()

    with tile.TileContext(nc) as tc, ExitStack() as ctx:
        const = ctx.enter_context(tc.tile_pool(name="const", bufs=1))
        state = ctx.enter_context(tc.tile_pool(name="state", bufs=1))
        xin = ctx.enter_context(tc.tile_pool(name="xin", bufs=2))
        outp = ctx.enter_context(tc.tile_pool(name="outp", bufs=2))
        ps1 = ctx.enter_context(tc.tile_pool(name="ps1", bufs=3, space="PSUM"))
        ps2 = ctx.enter_context(tc.tile_pool(name="ps2", bufs=3, space="PSUM"))

        w1a_s = const.tile([KA, H], F16, tag="w1a")
        w1c_s = const.tile([KC, H], F16, tag="w1c")
        nhi_s = const.tile([H, H], F16, tag="nhi")
        ni5_s = const.tile([O, O], F16, tag="ni5")
        w2hi_s = const.tile([H, O], F16, tag="w2hi")
        w2lo_s = const.tile([H, O], F16, tag="w2lo")
        rho_s = const.tile([O, 1], F32, tag="rho")
        for s, d in [
            (w1a_s, w1a_d),
            (w1c_s, w1c_d),
            (nhi_s, nhi_d),
            (ni5_s, ni5_d),
            (w2hi_s, w2hi_d),
            (w2lo_s, w2lo_d),
            (rho_s, rho_d),
        ]:
            nc.sync.dma_start(out=s[:], in_=d[:])

        # Recurrent state, ping-pong buffered (index = t % 2).
        z1 = [state.tile([H, BC], F32, tag=f"z1_{p}", name=f"z1_{p}") for p in range(2)]
        sg = [state.tile([H, BC], F16, tag=f"sg_{p}", name=f"sg_{p}") for p in range(2)]
        y2 = [state.tile([O, BC], F32, tag=f"y2_{p}", name=f"y2_{p}") for p in range(2)]
        sg2init = state.tile([O, BC], F16, tag="sg2init")

        nc.vector.memset(z1[1][:], -1.0)  # m1(0)=0 -> z1=-1
        nc.vector.memset(sg[1][:], -1.0)  # sign(-1)
        nc.sync.dma_start(out=y2[1][:], in_=y2i_d[:])
        nc.vector.memset(sg2init[:], -1.0)

        # x-chunk DMA row split across the 3 DMA-capable queues
        dma_engines = [nc.sync, nc.gpsimd, nc.scalar]
        row_splits = [0, 27, 54, KC]
        rl = ch * BC  # elements per row in a chunk
        n_chunks = n_steps // ch

        def fetch_chunk(chk):
            xt = xin.tile([KC, ch * BC], F16, tag="xt", name=f"xt{chk}")
            for q in range(3):
                r0, r1 = row_splits[q], row_splits[q + 1]
                dma_engines[q].dma_start(
                    out=xt[r0:r1, :],
                    in_=x_d[chk : chk + 1, r0 * rl : r1 * rl],
                )
            return xt

        def mm1_xpart(p1, xt, st):
            """The two x-only matmuls opening psum group for one step."""
            xs_a = xt[:KA, st * BC : (st + 1) * BC]
            xs_c = xt[:KC, st * BC : (st + 1) * BC]
            nc.tensor.matmul(p1[:], w1a_s[:], xs_a, start=True, stop=False)
            nc.tensor.matmul(p1[:], w1c_s[:], xs_c, start=False, stop=False)

        # Software-pipelined loop: PE work for step t+1's x-part is emitted
        # right after step t's sign so the PE never head-of-line blocks on
        # the recurrent dependency.
        xts = [fetch_chunk(0)]
        ot = None
        sg2_prev = sg2init[:]
        p1 = ps1.tile([H, BC], F32, tag="p1")
        mm1_xpart(p1, xts[0], 0)
        for t in range(n_steps):
            chk, st = divmod(t, ch)
            if st == 0:
                if chk + 1 < n_chunks:
                    xts.append(fetch_chunk(chk + 1))
                ot = outp.tile([O, ch * BC], F16, tag="ot")
            cur, prv = t % 2, 1 - (t % 2)

            # ---- layer 1: close step t's psum group, update, spike ----
            nc.tensor.matmul(p1[:], nhi_s[:], sg[prv][:], start=False, stop=True)
            nc.vector.scalar_tensor_tensor(
                z1[cur][:], z1[prv][:], BETA, p1[:], MULT, ADD
            )
            nc.scalar.sign(sg[cur][:], z1[cur][:])

            # ---- open step t+1's psum group (x-only, independent) ----
            if t + 1 < n_steps:
                nchk, nst = divmod(t + 1, ch)
                p1 = ps1.tile([H, BC], F32, tag="p1")
                mm1_xpart(p1, xts[nchk], nst)

            # ---- layer 2 ([5, 512]) ----
            p2 = ps2.tile([O, BC], F32, tag="p2")
            nc.tensor.matmul(p2[:], w2hi_s[:], sg[cur][:], start=True, stop=False)
            nc.tensor.matmul(p2[:], w2lo_s[:], sg[cur][:], start=False, stop=False)
            nc.tensor.matmul(p2[:], ni5_s[:], sg2_prev, start=False, stop=True)
            nc.vector.scalar_tensor_tensor(
                y2[cur][:], y2[prv][:], BETA, p2[:], MULT, ADD
            )
            o_slice = ot[:, st * BC : (st + 1) * BC]
            nc.scalar.sign(o_slice, y2[cur][:], bias=rho_s[:])
            sg2_prev = o_slice

            if st == ch - 1:
                nc.sync.dma_start(
                    out=out_d[:, chk * ch * BC : (chk + 1) * ch * BC], in_=ot[:]
                )


def _split16(a):
    hi = a.astype(np.float16)
    lo = (a.astype(np.float32) - hi.astype(np.float32)).astype(np.float16)
    return hi, lo


def host_inputs(x, W1, b1, W2, b2, n_steps=T, ch=CH):
    """Shard + precompute all per-core device input arrays."""
    n_chunks = n_steps // ch
    x = np.asarray(x, np.float32)[:, :n_steps, :]
    W1 = np.asarray(W1, np.float32)
    b1 = np.asarray(b1, np.float32)
    W2 = np.asarray(W2, np.float32)
    b2 = np.asarray(b2, np.float32)

    # x: [B, T', F] -> per core [T', F, 512] hi/lo-split + ones row
    xs = x.reshape(NCORES, BC, n_steps, F).transpose(0, 2, 3, 1)  # [8,T',40,512]
    xhi, xlo = _split16(xs)
    aug = np.empty((NCORES, n_steps, KC, BC), np.float16)
    aug[:, :, :F, :] = xhi
    aug[:, :, F, :] = 1.0
    aug[:, :, F + 1 :, :] = xlo
    # -> [8, n_chunks, 81, ch, 512] (chunk-major, partition dim 81)
    aug = aug.reshape(NCORES, n_chunks, ch, KC, BC).transpose(0, 1, 3, 2, 4)
    aug = np.ascontiguousarray(aug).reshape(NCORES, n_chunks, KC * ch * BC)

    w1hi, w1lo = _split16(W1.T)  # [40, 128]
    bias1 = b1 + BETA - 1.5
    bhi, blo = _split16(bias1)
    w1a = np.concatenate([w1hi, bhi[None, :]], axis=0)  # [41, 128]
    w1c = np.concatenate([w1lo, blo[None, :], w1hi], axis=0)  # [81, 128]

    neg_half_i = (-0.5 * np.eye(H)).astype(np.float16)
    neg_i5 = (-0.5 * np.eye(O)).astype(np.float16)
    w2hi, w2lo = _split16(0.5 * W2.T)  # [128, 5]

    c2 = 0.5 * W2.sum(axis=1) + b2 + BETA - 1.5
    rho = (c2 / (1.0 - BETA)).astype(np.float32)
    y2init = np.tile((-1.0 - rho)[:, None], (1, BC)).astype(np.float32)

    shared = {
        "w1a": np.ascontiguousarray(w1a),
        "w1c": np.ascontiguousarray(w1c),
        "neg_half_i": neg_half_i,
        "neg_i5": neg_i5,
        "w2hi": np.ascontiguousarray(w2hi),
        "w2lo": np.ascontiguousarray(w2lo),
        "rho": rho[:, None].copy(),
        "y2init": y2init,
    }
    return [{"x_aug": aug[c], **shared} for c in range(NCORES)]


def assemble(results, n_steps=T):
    """[O, T'*BC] fp16 sign form per core -> [T', B, O] float32 spikes."""
    outs = []
    for r in results:
        a = np.asarray(r["out"]).reshape(O, n_steps, BC)
        outs.append((a > 0).transpose(1, 2, 0).astype(np.float32))  # [T', 512, 5]
    return np.concatenate(outs, axis=1)


LAST_RESULT = None  # BassKernelResults of the most recent run (for profiling)


def kernel(x, W1, b1, W2, b2):
    global LAST_RESULT
    in_maps = host_inputs(x, W1, b1, W2, b2)
    nc = bacc.Bacc("TRN2", target_bir_lowering=False, debug=False)
    build(nc)
    nc.compile()
    LAST_RESULT = run_bass_kernel_spmd(nc, in_maps, list(range(NCORES)))
    return assemble(LAST_RESULT.results)
